# revision 1
# baseline (speedup 1.0000x reference)
"""Trainium2 Bass kernel: fused concat-linear attention map + softmax.

reference:  scores[b,h,n] = key[b,n,:]@Wk[h,:] + query[b,0,:]@Wq[h,:] + bias[h]
            attn = softmax over n              (B=16, N=20000, D=256, H=8)

Sharding: batch dim B=16 split across 8 cores (2 batches/core), weights
replicated.  Per batch the kernel streams key (20.5 MB f32) through:
  DMA (2 MB chunks, natural layout)
    -> PE transpose of 128x128 blocks (exact data movement; gives d on
       partitions, which the matmul contraction requires)
    -> DVE/ACT copy PSUM->SBUF
    -> PE matmul vs tiny stationary WkT [128,8] (float32r moving rate:
       1 cycle/row) accumulating the two d-halves in PSUM
    -> ScalarE fused exp(x + (qWq+b)[h]) with per-chunk accumulated sums
       (softmax without max-subtraction: scores are O(+-7) so f32 exp is
       safe and mathematically identical)
    -> DVE scale by 1/sum, contiguous DMA out.
"""

import sys

import numpy as np

for _p in ("/opt/trn_rl_repo",):
    if _p not in sys.path:
        sys.path.append(_p)

from contextlib import ExitStack

import concourse.bass as bass
import concourse.bacc as bacc
import concourse.tile as tile
from concourse import mybir
from concourse.masks import make_identity

B, N, D, H = 16, 20000, 256, 8
NCORES = 8
BPC = B // NCORES  # batches per core
P = 128
CHUNK = 512  # n-columns per score chunk (= one PSUM bank of f32)
LOAD_SUB = 16  # 128-row subtiles per load DMA (2048 rows = 2 MB)
F32 = mybir.dt.float32
F32R = mybir.dt.float32r


def _ceil_div(a, b):
    return (a + b - 1) // b


def build_kernel(n=N, bpc=BPC, score_dtype=F32R, tp_dtype=F32):
    nc = bacc.Bacc("TRN2", target_bir_lowering=False, debug=False)
    q_in = nc.declare_dram_parameter("q", [bpc, D], F32, isOutput=False)
    k_in = nc.declare_dram_parameter("k", [bpc, n, D], F32, isOutput=False)
    w_in = nc.declare_dram_parameter("w", [H, 2 * D], F32, isOutput=False)
    b_in = nc.declare_dram_parameter("b", [H], F32, isOutput=False)
    out = nc.declare_dram_parameter("out", [bpc, H, n], F32, isOutput=True)

    nchunks = _ceil_div(n, CHUNK)

    with ExitStack() as ctx:
        tc = ctx.enter_context(tile.TileContext(nc))
        consts = ctx.enter_context(tc.tile_pool(name="consts", bufs=1))
        loads = ctx.enter_context(tc.tile_pool(name="loads", bufs=3))
        kts = ctx.enter_context(tc.tile_pool(name="kts", bufs=3))
        probp = ctx.enter_context(tc.tile_pool(name="prob", bufs=1))
        small = ctx.enter_context(tc.tile_pool(name="small", bufs=2))
        psum_kt = ctx.enter_context(tc.tile_pool(name="psum_kt", bufs=2, space="PSUM"))
        psum_sc = ctx.enter_context(tc.tile_pool(name="psum_sc", bufs=2, space="PSUM"))
        psum_mi = ctx.enter_context(tc.tile_pool(name="psum_mi", bufs=1, space="PSUM"))

        identity = consts.tile([P, P], F32)
        make_identity(nc, identity)
        id_t = identity[:, :] if tp_dtype == F32 else identity[:, :].bitcast(tp_dtype)

        # --- constants: W transposed to [d, h] chunks, bias, queries -------
        w_sb = consts.tile([H, 2 * D], F32)
        nc.sync.dma_start(out=w_sb[:, :], in_=w_in[:, :])
        b_sb = consts.tile([H, 1], F32)
        nc.sync.dma_start(out=b_sb[:, :], in_=b_in[:])

        # wqT[:, c, :]: WqT halves (exact f32); wkT[:, c, :]: WkT halves,
        # rounded to the score matmul dtype during the PSUM->SBUF copy.
        wqT = consts.tile([P, 2, H], F32)
        wkT = consts.tile([P, 2, H], score_dtype)
        for c in range(4):
            pt = psum_mi.tile([P, H], F32, tag="mi")
            nc.tensor.transpose(pt[:, :], w_sb[:, c * P:(c + 1) * P], identity[:H, :H])
            dst = wqT[:, c, :] if c < 2 else wkT[:, c - 2, :]
            nc.vector.tensor_copy(out=dst, in_=pt[:, :])

        q_sb = consts.tile([1, bpc, D], F32)
        nc.sync.dma_start(out=q_sb[:, :, :], in_=q_in[:, :])
        qT = consts.tile([P, bpc, 2], F32)
        for i in range(bpc):
            for c in range(2):
                pt = psum_mi.tile([P, 1], F32, tag="mi")
                nc.tensor.transpose(
                    pt[:, :], q_sb[0:1, i, c * P:(c + 1) * P], identity[:1, :1]
                )
                nc.vector.tensor_copy(out=qT[:, i, c:c + 1], in_=pt[:, :])

        # qb[:, i] = Wq @ q_i + b   (full-f32 matmul; 1-row stream, trivial)
        qb = consts.tile([H, bpc], F32)
        for i in range(bpc):
            qp = psum_mi.tile([H, 1], F32, tag="mi")
            nc.tensor.matmul(
                qp[:, :], wqT[:, 0, :], qT[:, i, 0:1], start=True, stop=False
            )
            nc.tensor.matmul(
                qp[:, :], wqT[:, 1, :], qT[:, i, 1:2], start=False, stop=True
            )
            nc.vector.tensor_add(qb[:, i:i + 1], qp[:, :], b_sb[:, :])

        wk0 = wkT[:, 0, :]
        wk1 = wkT[:, 1, :]

        # --- main loop ------------------------------------------------------
        load_rows = LOAD_SUB * P
        nloads = _ceil_div(n, load_rows)
        for i in range(bpc):
            prob = probp.tile([H, n], F32, tag="prob")
            sums = small.tile([H, nchunks], F32, tag="sums")
            gchunk = 0
            for L in range(nloads):
                n0 = L * load_rows
                rows = min(load_rows, n - n0)
                full_sub = rows // P
                rem = rows - full_sub * P
                ld = loads.tile([P, LOAD_SUB, D], F32, tag="load")
                if full_sub:
                    nc.sync.dma_start(
                        out=ld[:, :full_sub, :],
                        in_=k_in[i, n0:n0 + full_sub * P, :].rearrange(
                            "(s p) d -> p s d", p=P
                        ),
                    )
                if rem:
                    nc.sync.dma_start(
                        out=ld[:rem, full_sub, :],
                        in_=k_in[i, n0 + full_sub * P:n0 + rows, :],
                    )
                for sc in range(_ceil_div(rows, CHUNK)):
                    w = min(CHUNK, rows - sc * CHUNK)
                    nsub = _ceil_div(w, P)
                    kt0 = psum_kt.tile([P, CHUNK], F32, tag="kt0")
                    kt1 = psum_kt.tile([P, CHUNK], F32, tag="kt1")
                    def _tp(ap):
                        return ap if tp_dtype == F32 else ap.bitcast(tp_dtype)

                    for t in range(nsub):
                        tw = min(P, w - t * P)
                        s = sc * (CHUNK // P) + t
                        nc.tensor.transpose(
                            _tp(kt0[:, t * P:t * P + tw]),
                            _tp(ld[:tw, s, 0:P]),
                            id_t[:tw, :tw],
                        )
                        nc.tensor.transpose(
                            _tp(kt1[:, t * P:t * P + tw]),
                            _tp(ld[:tw, s, P:2 * P]),
                            id_t[:tw, :tw],
                        )
                    k0 = kts.tile([P, CHUNK], score_dtype, tag="k0")
                    k1 = kts.tile([P, CHUNK], score_dtype, tag="k1")
                    # alternate engines so PSUM->SBUF copy load is split
                    def _copy_v(o, s):
                        nc.vector.tensor_copy(out=o, in_=s)

                    def _copy_a(o, s):
                        nc.scalar.copy(out=o, in_=s)

                    eng_a = _copy_v if gchunk % 2 == 0 else _copy_a
                    eng_b = _copy_a if gchunk % 2 == 0 else _copy_v
                    eng_a(k0[:, :w], kt0[:, :w])
                    eng_b(k1[:, :w], kt1[:, :w])
                    scp = psum_sc.tile([H, CHUNK], F32, tag="sc")
                    nc.tensor.matmul(
                        scp[:, :w], wk0, k0[:, :w], start=True, stop=False
                    )
                    nc.tensor.matmul(
                        scp[:, :w], wk1, k1[:, :w], start=False, stop=True
                    )
                    nc.scalar.activation(
                        out=prob[:, n0 + sc * CHUNK:n0 + sc * CHUNK + w],
                        in_=scp[:, :w],
                        func=mybir.ActivationFunctionType.Exp,
                        bias=qb[:, i:i + 1],
                        scale=1.0,
                        accum_out=sums[:, gchunk:gchunk + 1],
                    )
                    gchunk += 1
            assert gchunk == nchunks
            tot = small.tile([H, 1], F32, tag="tot")
            nc.vector.reduce_sum(out=tot[:, :], in_=sums[:, :], axis=mybir.AxisListType.X)
            rec = small.tile([H, 1], F32, tag="rec")
            nc.vector.reciprocal(out=rec[:, :], in_=tot[:, :])
            nc.vector.tensor_scalar_mul(prob[:, :], prob[:, :], rec[:, :])
            nc.sync.dma_start(out=out[i, :, :], in_=prob[:, :])

    nc.compile()
    return nc


_NC_CACHE = {}


def _get_nc():
    if "nc" not in _NC_CACHE:
        _NC_CACHE["nc"] = build_kernel()
    return _NC_CACHE["nc"]


def kernel(query, key, W, b):
    from concourse.bass_utils import run_bass_kernel_spmd

    query = np.ascontiguousarray(np.asarray(query, np.float32).reshape(B, D))
    key = np.ascontiguousarray(np.asarray(key, np.float32))
    W = np.ascontiguousarray(np.asarray(W, np.float32))
    b = np.ascontiguousarray(np.asarray(b, np.float32))

    nc = _get_nc()
    in_maps = []
    for c in range(NCORES):
        s = slice(BPC * c, BPC * (c + 1))
        in_maps.append(
            {
                "q": query[s],
                "k": key[s],
                "w": W,
                "b": b,
            }
        )
    res = run_bass_kernel_spmd(nc, in_maps, list(range(NCORES))).results
    return np.concatenate([res[c]["out"] for c in range(NCORES)], axis=0)

